# revision 26
# baseline (speedup 1.0000x reference)
"""Trainium2 Bass kernel for LeNet-C3 binarized 5x5 VALID conv.

out[256,16,124,124] = conv2d(x[256,6,128,128], sign(W)*mask), NCHW/OIHW.

Strategy (per core, data-parallel over batch, 8 cores x 32 images):
  Parity-split 3-pass matmul scheme, bf16 operands, f32 PSUM accumulation.

  Split w by parity: x[ci, h, 2u+c] -> partition (ci, r, c), free u.
  Per output row-block b (j=4 rows, 31 blocks), input rows 4b..4b+7:
    K = (ci, r in 8, c in 2) = 96 partitions
    M = (co 16, j 4, v 2)   = 128  -> output (co, 4b+j, 2*u0+v)
    3 PSUM-accumulated matmuls (pass p streams free offset u0+p):
      ps[(co,j,v), (n,u0)] += S_p[(ci,r,c),(co,j,v)]^T @ xb[(ci,r,c),(n,u0+p)]
    with S_p[...] = wb[co, ci, r-j, 2p+c-v] (zero where kh/kw out of range).

  vs the 5-pass f32 baseline this is 1.67x fewer PE columns (3*62 vs 5*124
  per 8-image group covering 2x w per column) and ~1.7x less HBM traffic
  (bf16 both directions; input rows 2x-replicated by the 8-row/4-stride
  blocking, outputs written exactly once).

  DMA layouts are partition-major so every transfer is contiguous per
  partition row: x dram [96, 31*2048], o dram [128, 31*1984].
"""

import sys

sys.path.insert(0, "/opt/trn_rl_repo")

import numpy as np

# ---- problem constants (hardcoded per contract) ----
N_CORES = 8
N, CI, H, WI = 256, 6, 128, 128
CO, KH, KW = 16, 5, 5
HO, WO = 124, 124
NPC = N // N_CORES  # images per core (32)

JB = 4              # output rows per block
VB = 2              # output w per psum column (parity pair)
RB = JB + KH - 1    # input rows per block (8)
KP = CI * RB * 2    # contraction partitions (96)
NBLK = HO // JB     # 31 row blocks
NU = WI // 2        # 64 u positions per image
UO = WO // 2        # 62 psum columns per image
NSUB = 8            # images per matmul tile (moving N = 8*62 = 496 <= 1024)
NGRP = NPC // NSUB  # 4 matmul groups per block
NPASS = 3
CB = 2              # row blocks per input DMA chunk (last chunk may be short)
SB = 2              # row blocks per output store
ALT_RINGS = True    # alternate both loads and stores across both HWDGE rings
STORE_GPSIMD = False  # route stores through SWDGE (third DMA path): slower
S_MAJOR = True      # stationary-major matmul ordering (fewer weight switches)
STORE_LAG = 0       # delay store issue by this many blocks (avoids ring
                    # head-of-line blocking on unfinished PSUM copies)

FEATURE_MAPS = [
    [0, 1, 2], [1, 2, 3], [2, 3, 4], [3, 4, 5], [0, 4, 5], [0, 1, 5],
    [0, 1, 2, 3], [1, 2, 3, 4], [2, 3, 4, 5], [0, 3, 4, 5], [0, 1, 4, 5],
    [0, 1, 2, 5], [0, 1, 3, 4], [1, 2, 4, 5], [0, 2, 3, 5],
    [0, 1, 2, 3, 4, 5],
]


def _channel_mask():
    m = np.zeros((CO, CI, 1, 1), np.float32)
    for i, maps in enumerate(FEATURE_MAPS):
        m[i, maps, 0, 0] = 1.0
    return m


def _np_bf16():
    import ml_dtypes

    return ml_dtypes.bfloat16


def _build_stationary(wb):
    """S[p, (ci,r,c), (co,j,v)] = wb[co, ci, r-j, 2p+c-v] where valid."""
    S = np.zeros((NPASS, KP, 128), np.float32)
    for p in range(NPASS):
        for ci in range(CI):
            for r in range(RB):
                for c in range(2):
                    krow = ci * 16 + r * 2 + c
                    for co in range(CO):
                        for j in range(JB):
                            kh = r - j
                            if not (0 <= kh < KH):
                                continue
                            for v in range(VB):
                                kw = 2 * p + c - v
                                if 0 <= kw < KW:
                                    S[p, krow, co * 8 + j * 2 + v] = wb[
                                        co, ci, kh, kw
                                    ]
    return S


def _pack_x(shard):
    """[npc, CI, H, WI] f32 -> [KP, NBLK*npc*NU] bf16 partition-major."""
    npc = shard.shape[0]
    xv = shard.transpose(1, 2, 0, 3)  # [ci, h, n, w]
    xb = np.empty((NBLK, CI, RB, 2, npc, NU), np.float32)
    for b in range(NBLK):
        rows = xv[:, 4 * b: 4 * b + RB]               # [ci, r, n, w]
        xb[b] = rows.reshape(CI, RB, npc, NU, 2).transpose(0, 1, 4, 2, 3)
    # -> partition (ci, r, c), free (b, n, u)
    xp = xb.transpose(1, 2, 3, 0, 4, 5).reshape(KP, NBLK * npc * NU)
    return xp.astype(_np_bf16())


def _unpack_o(o_np, npc):
    """[128, NBLK*npc*UO] bf16 -> [npc, CO, HO, WO] f32."""
    blocks = np.asarray(o_np, dtype=np.float32).reshape(
        CO, JB, VB, NBLK, npc, UO
    )
    # out[n, co, 4b+j, 2u+v]
    out = blocks.transpose(4, 0, 3, 1, 5, 2).reshape(npc, CO, HO, WO)
    return np.ascontiguousarray(out)


def _body(
    nc,
    x,
    o,
    st,
    xpool,
    opool,
    ppool,
    npc,
    do_load=True,
    do_mm=True,
    do_copy=True,
    do_store=True,
    xfix=None,
    obfix=None,
):
    import concourse.mybir as mybir

    f32 = mybir.dt.float32
    bf16 = mybir.dt.bfloat16

    chunk_starts = list(range(0, NBLK, CB))

    def issue_load(idx):
        cs = chunk_starts[idx]
        nblk_c = min(CB, NBLK - cs)
        xt = xpool.tile([KP, CB, npc, NU], bf16, tag="xt")
        leng = (nc.sync, nc.scalar)[idx % 2 if ALT_RINGS else 0]
        leng.dma_start(
            xt[:, 0:nblk_c, :, :].rearrange("p b n u -> p (b n u)"),
            x[:, cs * npc * NU: (cs + nblk_c) * npc * NU],
        )
        return xt

    PREFETCH = 2
    xts = {}
    if do_load:
        for i in range(min(PREFETCH, len(chunk_starts))):
            xts[i] = issue_load(i)

    ob = None
    n_store = [0]
    store_q = []

    def flush_store():
        b0, nsb, obt = store_q.pop(0)
        if STORE_GPSIMD:
            seng = nc.gpsimd
        else:
            seng = (nc.scalar, nc.sync)[n_store[0] % 2 if ALT_RINGS else 0]
        n_store[0] += 1
        seng.dma_start(
            o[:, b0 * npc * UO: (b0 + nsb) * npc * UO],
            obt[:, 0:nsb, :, :].rearrange("p b n u -> p (b n u)"),
        )

    for b in range(NBLK):
        ci_, bb = divmod(b, CB)
        if do_load:
            if bb == 0:
                if ci_ + PREFETCH < len(chunk_starts):
                    xts[ci_ + PREFETCH] = issue_load(ci_ + PREFETCH)
                if ci_ - 1 in xts:
                    del xts[ci_ - 1]
            xt = xts[ci_]
        else:
            xt = xfix
            bb = 0
        sb = b % SB
        if do_copy and sb == 0:
            ob = opool.tile([128, SB, npc, UO], bf16, tag="ob")
        elif not do_copy:
            ob = obfix
        if do_mm:
            if S_MAJOR:
                pss = [
                    ppool.tile([128, NSUB, UO], f32, tag="ps", name=f"ps{ng}")
                    for ng in range(NGRP)
                ]
                for p in range(NPASS):
                    for ng in range(NGRP):
                        n0 = ng * NSUB
                        nc.tensor.matmul(
                            pss[ng][:],
                            st[:, p, :],
                            xt[:, bb, n0: n0 + NSUB, p: p + UO],
                            start=(p == 0),
                            stop=(p == NPASS - 1),
                        )
                if do_copy:
                    for ng in range(NGRP):
                        n0 = ng * NSUB
                        nc.vector.tensor_copy(
                            ob[:, sb, n0: n0 + NSUB, :], pss[ng][:]
                        )
            else:
                for ng0 in range(0, NGRP, 2):
                    n0a, n0b = ng0 * NSUB, (ng0 + 1) * NSUB
                    psa = ppool.tile([128, NSUB, UO], f32, tag="ps")
                    psb = ppool.tile([128, NSUB, UO], f32, tag="ps")
                    for p in range(NPASS):
                        nc.tensor.matmul(
                            psa[:],
                            st[:, p, :],
                            xt[:, bb, n0a: n0a + NSUB, p: p + UO],
                            start=(p == 0),
                            stop=(p == NPASS - 1),
                        )
                        nc.tensor.matmul(
                            psb[:],
                            st[:, p, :],
                            xt[:, bb, n0b: n0b + NSUB, p: p + UO],
                            start=(p == 0),
                            stop=(p == NPASS - 1),
                        )
                    if do_copy:
                        nc.vector.tensor_copy(
                            ob[:, sb, n0a: n0a + NSUB, :], psa[:]
                        )
                        nc.vector.tensor_copy(
                            ob[:, sb, n0b: n0b + NSUB, :], psb[:]
                        )
        if do_store and (sb == SB - 1 or b == NBLK - 1):
            store_q.append((b - sb, sb + 1, ob))
            while len(store_q) > STORE_LAG:
                flush_store()
    if do_store:
        while store_q:
            flush_store()


def build_nc(npc=NPC, reps=1):
    import concourse.mybir as mybir
    import concourse.tile as tile
    from concourse import bacc

    bf16 = mybir.dt.bfloat16

    nc = bacc.Bacc(None, target_bir_lowering=False)
    x = nc.dram_tensor(
        "x", [KP, NBLK * npc * NU], bf16, kind="ExternalInput"
    )
    s = nc.dram_tensor("s", [KP, NPASS, 128], bf16, kind="ExternalInput")
    o = nc.dram_tensor(
        "o", [128, NBLK * npc * UO], bf16, kind="ExternalOutput"
    )

    with tile.TileContext(nc) as tc:
        with (
            tc.tile_pool(name="spool", bufs=1) as spool,
            tc.tile_pool(name="xpool", bufs=4) as xpool,
            tc.tile_pool(name="opool", bufs=6) as opool,
            tc.tile_pool(name="ppool", bufs=8, space="PSUM") as ppool,
        ):
            st = spool.tile([KP, NPASS, 128], bf16)
            nc.sync.dma_start(st[:], s[:, :, :].rearrange("p a m -> p (a m)"))
            for _rep in range(reps):
                _body(nc, x, o, st, xpool, opool, ppool, npc)
    nc.compile()
    return nc


def _timing_shell(
    npc, reps, body_fn, staggered_reset=False, unroll=1, count=True,
    fixtures=True,
):
    """Common For_i timing harness: internal DRAM output + rep counter."""
    import concourse.mybir as mybir
    import concourse.tile as tile
    from concourse import bacc

    f32 = mybir.dt.float32
    bf16 = mybir.dt.bfloat16
    ET = mybir.EngineType

    nc = bacc.Bacc(None, target_bir_lowering=False)
    x = nc.dram_tensor(
        "x", [KP, NBLK * npc * NU], bf16, kind="ExternalInput"
    )
    s = nc.dram_tensor("s", [KP, NPASS, 128], bf16, kind="ExternalInput")
    t = nc.dram_tensor("t", [1, 1], f32, kind="ExternalOutput")

    with tile.TileContext(nc) as tc:
        with (
            tc.tile_pool(name="spool", bufs=1) as spool,
            tc.tile_pool(name="xpool", bufs=4) as xpool,
            tc.tile_pool(name="opool", bufs=6) as opool,
            tc.tile_pool(name="ppool", bufs=8, space="PSUM") as ppool,
            tc.tile_pool(name="dpool", bufs=1, space="DRAM") as dpool,
        ):
            o = dpool.tile([128, NBLK * npc * UO], bf16)
            st = spool.tile([KP, NPASS, 128], bf16)
            nc.sync.dma_start(st[:], s[:, :, :].rearrange("p a m -> p (a m)"))
            if fixtures:
                xfix = spool.tile([KP, 1, npc, NU], bf16, tag="xfix")
                nc.sync.dma_start(
                    xfix[:].rearrange("p b n u -> p (b n u)"),
                    x[:, 0: npc * NU],
                )
                obfix = spool.tile([128, SB, npc, UO], bf16, tag="obfix")
                nc.gpsimd.memset(obfix[:], 0.25)
            else:
                xfix = obfix = None

            tb = spool.tile([1, 1], f32)
            nc.gpsimd.memset(tb[:], 1.0)
            tzero = spool.tile([1, 1], f32)
            nc.gpsimd.memset(tzero[:], 0.0)
            nc.sync.dma_start(t[:, :], tzero[:])

            def body():
                body_fn(nc, x, o, st, xpool, opool, ppool, xfix, obfix)
                if count:
                    nc.gpsimd.dma_start(
                        t[:, :], tb[:], accum_op=mybir.AluOpType.add
                    )

            if reps == 1:
                body()
            else:
                with tc.For_i(
                    0,
                    (reps - 1) // unroll,
                    1,
                    hint_engines=(ET.PE, ET.Activation, ET.DVE, ET.Pool, ET.SP),
                    staggered_reset=staggered_reset,
                ):
                    body()
    nc.compile()
    return nc


def build_nc_timing(reps, npc=NPC):
    def body_fn(nc, x, o, st, xpool, opool, ppool, xfix, obfix):
        _body(nc, x, o, st, xpool, opool, ppool, npc)

    return _timing_shell(npc, reps, body_fn)


def build_nc_micro(which, reps, npc=NPC):
    if which.startswith("u2"):
        which = which[2:]
        unroll = 2
    else:
        unroll = 1
    if which.startswith("sr"):
        which = which[2:]
        stag = True
    else:
        stag = False
    if which.endswith("_nc"):
        which = which[:-3]
        count = False
    else:
        count = True

    flags = {
        "mm": dict(do_load=False, do_copy=False, do_store=False),
        "mmcopy": dict(do_load=False, do_store=False),
        "load": dict(do_mm=False, do_copy=False, do_store=False),
        "store": dict(do_load=False, do_mm=False, do_copy=False),
        "nostore": dict(do_store=False),
        "mcs": dict(do_load=False),
        "lmst": dict(do_copy=False),
        "loadstore": dict(do_mm=False, do_copy=False),
        "full": dict(),
    }[which]

    def body_fn(nc, x, o, st, xpool, opool, ppool, xfix, obfix):
        for _ in range(unroll):
            _body(
                nc, x, o, st, xpool, opool, ppool, npc,
                xfix=xfix, obfix=obfix, **flags,
            )

    return _timing_shell(
        npc, reps, body_fn, staggered_reset=stag, unroll=unroll, count=count,
        fixtures=(which != "full"),
    )


_NC_CACHE = {}


def _get_nc(npc=NPC):
    if npc not in _NC_CACHE:
        _NC_CACHE[npc] = build_nc(npc)
    return _NC_CACHE[npc]


def make_in_maps(x, W):
    wb = (np.sign(W) * _channel_mask()).astype(np.float32)
    S = _build_stationary(wb).transpose(1, 0, 2).astype(_np_bf16())
    S = np.ascontiguousarray(S)  # [KP, NPASS, 128]
    shards = x.reshape(N_CORES, NPC, CI, H, WI)
    return [{"x": _pack_x(shards[i]), "s": S} for i in range(N_CORES)]


def _run(x, W, trace=False):
    from concourse.bass_utils import run_bass_kernel_spmd

    x = np.asarray(x, dtype=np.float32)
    W = np.asarray(W, dtype=np.float32)
    in_maps = make_in_maps(x, W)
    nc = _get_nc()
    res = run_bass_kernel_spmd(
        nc, in_maps, core_ids=list(range(N_CORES)), trace=trace
    )
    out = np.concatenate(
        [_unpack_o(r["o"], NPC) for r in res.results], axis=0
    )
    return out, res


def kernel(x, W):
    out, _ = _run(x, W, trace=False)
    return out


# revision 28
# speedup vs baseline: 1.0000x; 1.0000x over previous
"""Trainium2 Bass kernel for LeNet-C3 binarized 5x5 VALID conv.

out[256,16,124,124] = conv2d(x[256,6,128,128], sign(W)*mask), NCHW/OIHW.

Strategy (per core, data-parallel over batch, 8 cores x 32 images):
  Parity-split 3-pass matmul scheme, bf16 operands, f32 PSUM accumulation.

  Split w by parity: x[ci, h, 2u+c] -> partition (ci, r, c), free u.
  Per output row-block b (j=4 rows, 31 blocks), input rows 4b..4b+7:
    K = (ci, r in 8, c in 2) = 96 partitions
    M = (co 16, j 4, v 2)   = 128  -> output (co, 4b+j, 2*u0+v)
    3 PSUM-accumulated matmuls (pass p streams free offset u0+p):
      ps[(co,j,v), (n,u0)] += S_p[(ci,r,c),(co,j,v)]^T @ xb[(ci,r,c),(n,u0+p)]
    with S_p[...] = wb[co, ci, r-j, 2p+c-v] (zero where kh/kw out of range).

  vs the 5-pass f32 baseline this is 1.67x fewer PE columns (3*62 vs 5*124
  per 8-image group covering 2x w per column) and ~1.7x less HBM traffic
  (bf16 both directions; input rows 2x-replicated by the 8-row/4-stride
  blocking, outputs written exactly once).

  DMA layouts are partition-major so every transfer is contiguous per
  partition row: x dram [96, 31*2048], o dram [128, 31*1984].
"""

import sys

sys.path.insert(0, "/opt/trn_rl_repo")

import numpy as np

# ---- problem constants (hardcoded per contract) ----
N_CORES = 8
N, CI, H, WI = 256, 6, 128, 128
CO, KH, KW = 16, 5, 5
HO, WO = 124, 124
NPC = N // N_CORES  # images per core (32)

JB = 4              # output rows per block
VB = 2              # output w per psum column (parity pair)
RB = JB + KH - 1    # input rows per block (8)
KP = CI * RB * 2    # contraction partitions (96)
NBLK = HO // JB     # 31 row blocks
NU = WI // 2        # 64 u positions per image
UO = WO // 2        # 62 psum columns per image
NSUB = 8            # images per matmul tile (moving N = 8*62 = 496 <= 1024)
NGRP = NPC // NSUB  # 4 matmul groups per block
NPASS = 3
CB = 2              # row blocks per input DMA chunk (last chunk may be short)
SB = 2              # row blocks per output store
ALT_RINGS = True    # alternate both loads and stores across both HWDGE rings
STORE_GPSIMD = False  # route stores through SWDGE (third DMA path): slower
S_MAJOR = True      # stationary-major matmul ordering (fewer weight switches)
STORE_LAG = 0       # delay store issue by this many blocks (avoids ring
                    # head-of-line blocking on unfinished PSUM copies)

FEATURE_MAPS = [
    [0, 1, 2], [1, 2, 3], [2, 3, 4], [3, 4, 5], [0, 4, 5], [0, 1, 5],
    [0, 1, 2, 3], [1, 2, 3, 4], [2, 3, 4, 5], [0, 3, 4, 5], [0, 1, 4, 5],
    [0, 1, 2, 5], [0, 1, 3, 4], [1, 2, 4, 5], [0, 2, 3, 5],
    [0, 1, 2, 3, 4, 5],
]


def _channel_mask():
    m = np.zeros((CO, CI, 1, 1), np.float32)
    for i, maps in enumerate(FEATURE_MAPS):
        m[i, maps, 0, 0] = 1.0
    return m


def _np_bf16():
    import ml_dtypes

    return ml_dtypes.bfloat16


def _build_stationary(wb):
    """S[p, (ci,r,c), (co,j,v)] = wb[co, ci, r-j, 2p+c-v] where valid."""
    S = np.zeros((NPASS, KP, 128), np.float32)
    for p in range(NPASS):
        for ci in range(CI):
            for r in range(RB):
                for c in range(2):
                    krow = ci * 16 + r * 2 + c
                    for co in range(CO):
                        for j in range(JB):
                            kh = r - j
                            if not (0 <= kh < KH):
                                continue
                            for v in range(VB):
                                kw = 2 * p + c - v
                                if 0 <= kw < KW:
                                    S[p, krow, co * 8 + j * 2 + v] = wb[
                                        co, ci, kh, kw
                                    ]
    return S


def _pack_x(shard):
    """[npc, CI, H, WI] f32 -> [KP, NBLK*npc*NU] bf16 partition-major."""
    npc = shard.shape[0]
    xv = shard.transpose(1, 2, 0, 3)  # [ci, h, n, w]
    xb = np.empty((NBLK, CI, RB, 2, npc, NU), np.float32)
    for b in range(NBLK):
        rows = xv[:, 4 * b: 4 * b + RB]               # [ci, r, n, w]
        xb[b] = rows.reshape(CI, RB, npc, NU, 2).transpose(0, 1, 4, 2, 3)
    # -> partition (ci, r, c), free (b, n, u)
    xp = xb.transpose(1, 2, 3, 0, 4, 5).reshape(KP, NBLK * npc * NU)
    return xp.astype(_np_bf16())


def _unpack_o(o_np, npc):
    """[128, NBLK*npc*UO] bf16 -> [npc, CO, HO, WO] f32."""
    blocks = np.asarray(o_np, dtype=np.float32).reshape(
        CO, JB, VB, NBLK, npc, UO
    )
    # out[n, co, 4b+j, 2u+v]
    out = blocks.transpose(4, 0, 3, 1, 5, 2).reshape(npc, CO, HO, WO)
    return np.ascontiguousarray(out)


def _body(
    nc,
    x,
    o,
    st,
    xpool,
    opool,
    ppool,
    npc,
    do_load=True,
    do_mm=True,
    do_copy=True,
    do_store=True,
    xfix=None,
    obfix=None,
):
    import concourse.mybir as mybir

    f32 = mybir.dt.float32
    bf16 = mybir.dt.bfloat16

    chunk_starts = list(range(0, NBLK, CB))

    def issue_load(idx):
        cs = chunk_starts[idx]
        nblk_c = min(CB, NBLK - cs)
        xt = xpool.tile([KP, CB, npc, NU], bf16, tag="xt")
        leng = (nc.sync, nc.scalar)[idx % 2 if ALT_RINGS else 0]
        leng.dma_start(
            xt[:, 0:nblk_c, :, :].rearrange("p b n u -> p (b n u)"),
            x[:, cs * npc * NU: (cs + nblk_c) * npc * NU],
        )
        return xt

    PREFETCH = 2
    xts = {}
    if do_load:
        for i in range(min(PREFETCH, len(chunk_starts))):
            xts[i] = issue_load(i)

    ob = None
    n_store = [0]
    store_q = []

    def flush_store():
        b0, nsb, obt = store_q.pop(0)
        if STORE_GPSIMD:
            seng = nc.gpsimd
        else:
            seng = (nc.scalar, nc.sync)[n_store[0] % 2 if ALT_RINGS else 0]
        n_store[0] += 1
        seng.dma_start(
            o[:, b0 * npc * UO: (b0 + nsb) * npc * UO],
            obt[:, 0:nsb, :, :].rearrange("p b n u -> p (b n u)"),
        )

    for b in range(NBLK):
        ci_, bb = divmod(b, CB)
        if do_load:
            if bb == 0:
                if ci_ + PREFETCH < len(chunk_starts):
                    xts[ci_ + PREFETCH] = issue_load(ci_ + PREFETCH)
                if ci_ - 1 in xts:
                    del xts[ci_ - 1]
            xt = xts[ci_]
        else:
            xt = xfix
            bb = 0
        sb = b % SB
        if do_copy and sb == 0:
            ob = opool.tile([128, SB, npc, UO], bf16, tag="ob")
        elif not do_copy:
            ob = obfix
        if do_mm:
            if S_MAJOR:
                pss = [
                    ppool.tile([128, NSUB, UO], f32, tag="ps", name=f"ps{ng}")
                    for ng in range(NGRP)
                ]
                for p in range(NPASS):
                    for ng in range(NGRP):
                        n0 = ng * NSUB
                        nc.tensor.matmul(
                            pss[ng][:],
                            st[:, p, :],
                            xt[:, bb, n0: n0 + NSUB, p: p + UO],
                            start=(p == 0),
                            stop=(p == NPASS - 1),
                        )
                if do_copy:
                    for ng in range(NGRP):
                        n0 = ng * NSUB
                        nc.vector.tensor_copy(
                            ob[:, sb, n0: n0 + NSUB, :], pss[ng][:]
                        )
            else:
                for ng0 in range(0, NGRP, 2):
                    n0a, n0b = ng0 * NSUB, (ng0 + 1) * NSUB
                    psa = ppool.tile([128, NSUB, UO], f32, tag="ps")
                    psb = ppool.tile([128, NSUB, UO], f32, tag="ps")
                    for p in range(NPASS):
                        nc.tensor.matmul(
                            psa[:],
                            st[:, p, :],
                            xt[:, bb, n0a: n0a + NSUB, p: p + UO],
                            start=(p == 0),
                            stop=(p == NPASS - 1),
                        )
                        nc.tensor.matmul(
                            psb[:],
                            st[:, p, :],
                            xt[:, bb, n0b: n0b + NSUB, p: p + UO],
                            start=(p == 0),
                            stop=(p == NPASS - 1),
                        )
                    if do_copy:
                        nc.vector.tensor_copy(
                            ob[:, sb, n0a: n0a + NSUB, :], psa[:]
                        )
                        nc.vector.tensor_copy(
                            ob[:, sb, n0b: n0b + NSUB, :], psb[:]
                        )
        if do_store and (sb == SB - 1 or b == NBLK - 1):
            store_q.append((b - sb, sb + 1, ob))
            while len(store_q) > STORE_LAG:
                flush_store()
    if do_store:
        while store_q:
            flush_store()


def build_nc(npc=NPC, reps=1):
    import concourse.mybir as mybir
    import concourse.tile as tile
    from concourse import bacc

    bf16 = mybir.dt.bfloat16

    nc = bacc.Bacc(None, target_bir_lowering=False)
    x = nc.dram_tensor(
        "x", [KP, NBLK * npc * NU], bf16, kind="ExternalInput"
    )
    s = nc.dram_tensor("s", [KP, NPASS, 128], bf16, kind="ExternalInput")
    o = nc.dram_tensor(
        "o", [128, NBLK * npc * UO], bf16, kind="ExternalOutput"
    )

    with tile.TileContext(nc) as tc:
        with (
            tc.tile_pool(name="spool", bufs=1) as spool,
            tc.tile_pool(name="xpool", bufs=4) as xpool,
            tc.tile_pool(name="opool", bufs=6) as opool,
            tc.tile_pool(name="ppool", bufs=8, space="PSUM") as ppool,
        ):
            st = spool.tile([KP, NPASS, 128], bf16)
            nc.sync.dma_start(st[:], s[:, :, :].rearrange("p a m -> p (a m)"))
            for _rep in range(reps):
                _body(nc, x, o, st, xpool, opool, ppool, npc)
    nc.compile()
    return nc


def _timing_shell(
    npc, reps, body_fn, staggered_reset=False, unroll=1, count=True,
    fixtures=True,
):
    """Common For_i timing harness: internal DRAM output + rep counter."""
    import concourse.mybir as mybir
    import concourse.tile as tile
    from concourse import bacc

    f32 = mybir.dt.float32
    bf16 = mybir.dt.bfloat16
    ET = mybir.EngineType

    nc = bacc.Bacc(None, target_bir_lowering=False)
    x = nc.dram_tensor(
        "x", [KP, NBLK * npc * NU], bf16, kind="ExternalInput"
    )
    s = nc.dram_tensor("s", [KP, NPASS, 128], bf16, kind="ExternalInput")
    t = nc.dram_tensor("t", [1, 1], f32, kind="ExternalOutput")

    with tile.TileContext(nc) as tc:
        with (
            tc.tile_pool(name="spool", bufs=1) as spool,
            tc.tile_pool(name="xpool", bufs=4) as xpool,
            tc.tile_pool(name="opool", bufs=6) as opool,
            tc.tile_pool(name="ppool", bufs=8, space="PSUM") as ppool,
            tc.tile_pool(name="dpool", bufs=1, space="DRAM") as dpool,
        ):
            o = dpool.tile([128, NBLK * npc * UO], bf16)
            st = spool.tile([KP, NPASS, 128], bf16)
            nc.sync.dma_start(st[:], s[:, :, :].rearrange("p a m -> p (a m)"))
            if fixtures:
                xfix = spool.tile([KP, 1, npc, NU], bf16, tag="xfix")
                nc.sync.dma_start(
                    xfix[:].rearrange("p b n u -> p (b n u)"),
                    x[:, 0: npc * NU],
                )
                obfix = spool.tile([128, SB, npc, UO], bf16, tag="obfix")
                nc.gpsimd.memset(obfix[:], 0.25)
            else:
                xfix = obfix = None

            tb = spool.tile([1, 1], f32)
            nc.gpsimd.memset(tb[:], 1.0)
            tzero = spool.tile([1, 1], f32)
            nc.gpsimd.memset(tzero[:], 0.0)
            nc.sync.dma_start(t[:, :], tzero[:])

            def body():
                body_fn(nc, x, o, st, xpool, opool, ppool, xfix, obfix)
                if count:
                    nc.gpsimd.dma_start(
                        t[:, :], tb[:], accum_op=mybir.AluOpType.add
                    )

            if reps == 1:
                body()
            else:
                with tc.For_i(
                    0,
                    (reps - 1) // unroll,
                    1,
                    hint_engines=(ET.PE, ET.Activation, ET.DVE, ET.Pool, ET.SP),
                    staggered_reset=staggered_reset,
                ):
                    body()
    nc.compile()
    return nc


def build_nc_timing(reps, npc=NPC):
    def body_fn(nc, x, o, st, xpool, opool, ppool, xfix, obfix):
        _body(nc, x, o, st, xpool, opool, ppool, npc)

    return _timing_shell(npc, reps, body_fn)


def build_nc_micro(which, reps, npc=NPC):
    if which.startswith("u2"):
        which = which[2:]
        unroll = 2
    else:
        unroll = 1
    if which.startswith("sr"):
        which = which[2:]
        stag = True
    else:
        stag = False
    if which.endswith("_nc"):
        which = which[:-3]
        count = False
    else:
        count = True

    flags = {
        "mm": dict(do_load=False, do_copy=False, do_store=False),
        "mmcopy": dict(do_load=False, do_store=False),
        "load": dict(do_mm=False, do_copy=False, do_store=False),
        "store": dict(do_load=False, do_mm=False, do_copy=False),
        "nostore": dict(do_store=False),
        "mcs": dict(do_load=False),
        "lmst": dict(do_copy=False),
        "loadstore": dict(do_mm=False, do_copy=False),
        "full": dict(),
    }[which]

    def body_fn(nc, x, o, st, xpool, opool, ppool, xfix, obfix):
        for _ in range(unroll):
            _body(
                nc, x, o, st, xpool, opool, ppool, npc,
                xfix=xfix, obfix=obfix, **flags,
            )

    return _timing_shell(
        npc, reps, body_fn, staggered_reset=stag, unroll=unroll, count=count,
        fixtures=(which != "full"),
    )


_NC_CACHE = {}


def _get_nc(npc=NPC):
    if npc not in _NC_CACHE:
        _NC_CACHE[npc] = build_nc(npc)
    return _NC_CACHE[npc]


def make_in_maps(x, W):
    wb = (np.sign(W) * _channel_mask()).astype(np.float32)
    S = _build_stationary(wb).transpose(1, 0, 2).astype(_np_bf16())
    S = np.ascontiguousarray(S)  # [KP, NPASS, 128]
    shards = x.reshape(N_CORES, NPC, CI, H, WI)
    return [{"x": _pack_x(shards[i]), "s": S} for i in range(N_CORES)]


def _run(x, W, trace=False):
    from concourse.bass_utils import run_bass_kernel_spmd

    x = np.asarray(x, dtype=np.float32)
    W = np.asarray(W, dtype=np.float32)
    in_maps = make_in_maps(x, W)
    nc = _get_nc()
    res = run_bass_kernel_spmd(
        nc, in_maps, core_ids=list(range(N_CORES)), trace=trace
    )
    out = np.concatenate(
        [_unpack_o(r["o"], NPC) for r in res.results], axis=0
    )
    return out, res


def kernel(x, W):
    out, _ = _run(x, W, trace=False)
    return out


# revision 29
# speedup vs baseline: 1.1664x; 1.1664x over previous
"""Trainium2 Bass kernel for LeNet-C3 binarized 5x5 VALID conv.

out[256,16,124,124] = conv2d(x[256,6,128,128], sign(W)*mask), NCHW/OIHW.

Strategy (per core, data-parallel over batch, 8 cores x 32 images):
  Parity-split 3-pass matmul scheme, bf16 operands, f32 PSUM accumulation.

  Split w by parity: x[ci, h, 2u+c] -> partition (ci, r, c), free u.
  Per output row-block b (j=4 rows, 31 blocks), input rows 4b..4b+7:
    K = (ci, r in 8, c in 2) = 96 partitions
    M = (co 16, j 4, v 2)   = 128  -> output (co, 4b+j, 2*u0+v)
    3 PSUM-accumulated matmuls (pass p streams free offset u0+p):
      ps[(co,j,v), (n,u0)] += S_p[(ci,r,c),(co,j,v)]^T @ xb[(ci,r,c),(n,u0+p)]
    with S_p[...] = wb[co, ci, r-j, 2p+c-v] (zero where kh/kw out of range).

  vs the 5-pass f32 baseline this is 1.67x fewer PE columns (3*62 vs 5*124
  per 8-image group covering 2x w per column) and ~1.7x less HBM traffic
  (bf16 both directions; input rows 2x-replicated by the 8-row/4-stride
  blocking, outputs written exactly once).

  DMA layouts are partition-major so every transfer is contiguous per
  partition row: x dram [96, 31*2048], o dram [128, 31*1984].
"""

import sys

sys.path.insert(0, "/opt/trn_rl_repo")

import numpy as np

# ---- problem constants (hardcoded per contract) ----
N_CORES = 8
N, CI, H, WI = 256, 6, 128, 128
CO, KH, KW = 16, 5, 5
HO, WO = 124, 124
NPC = N // N_CORES  # images per core (32)

JB = 4              # output rows per block
VB = 2              # output w per psum column (parity pair)
RB = JB + KH - 1    # input rows per block (8)
KP = CI * RB * 2    # contraction partitions (96)
NBLK = HO // JB     # 31 row blocks
NU = WI // 2        # 64 u positions per image
UO = WO // 2        # 62 psum columns per image
NSUB = 8            # images per matmul tile (moving N = 8*62 = 496 <= 1024)
NGRP = NPC // NSUB  # 4 matmul groups per block
NPASS = 3
CB = 2              # row blocks per input DMA chunk (last chunk may be short)
SB = 2              # row blocks per output store
ALT_RINGS = True    # alternate both loads and stores across both HWDGE rings
STORE_GPSIMD = False  # route stores through SWDGE (third DMA path): slower
S_MAJOR = True      # stationary-major matmul ordering (fewer weight switches)
STORE_LAG = 0       # delay store issue by this many blocks (avoids ring
                    # head-of-line blocking on unfinished PSUM copies)

FEATURE_MAPS = [
    [0, 1, 2], [1, 2, 3], [2, 3, 4], [3, 4, 5], [0, 4, 5], [0, 1, 5],
    [0, 1, 2, 3], [1, 2, 3, 4], [2, 3, 4, 5], [0, 3, 4, 5], [0, 1, 4, 5],
    [0, 1, 2, 5], [0, 1, 3, 4], [1, 2, 4, 5], [0, 2, 3, 5],
    [0, 1, 2, 3, 4, 5],
]


def _channel_mask():
    m = np.zeros((CO, CI, 1, 1), np.float32)
    for i, maps in enumerate(FEATURE_MAPS):
        m[i, maps, 0, 0] = 1.0
    return m


def _np_bf16():
    import ml_dtypes

    return ml_dtypes.bfloat16


def _build_stationary(wb):
    """S[p, (ci,r,c), (co,j,v)] = wb[co, ci, r-j, 2p+c-v] where valid."""
    S = np.zeros((NPASS, KP, 128), np.float32)
    for p in range(NPASS):
        for ci in range(CI):
            for r in range(RB):
                for c in range(2):
                    krow = ci * 16 + r * 2 + c
                    for co in range(CO):
                        for j in range(JB):
                            kh = r - j
                            if not (0 <= kh < KH):
                                continue
                            for v in range(VB):
                                kw = 2 * p + c - v
                                if 0 <= kw < KW:
                                    S[p, krow, co * 8 + j * 2 + v] = wb[
                                        co, ci, kh, kw
                                    ]
    return S


def _pack_x(shard):
    """[npc, CI, H, WI] f32 -> [KP, NBLK*npc*NU] bf16 partition-major."""
    npc = shard.shape[0]
    xv = shard.transpose(1, 2, 0, 3)  # [ci, h, n, w]
    xb = np.empty((NBLK, CI, RB, 2, npc, NU), np.float32)
    for b in range(NBLK):
        rows = xv[:, 4 * b: 4 * b + RB]               # [ci, r, n, w]
        xb[b] = rows.reshape(CI, RB, npc, NU, 2).transpose(0, 1, 4, 2, 3)
    # -> partition (ci, r, c), free (b, n, u)
    xp = xb.transpose(1, 2, 3, 0, 4, 5).reshape(KP, NBLK * npc * NU)
    return xp.astype(_np_bf16())


def _unpack_o(o_np, npc):
    """[128, NBLK*npc*UO] bf16 -> [npc, CO, HO, WO] f32."""
    blocks = np.asarray(o_np, dtype=np.float32).reshape(
        CO, JB, VB, NBLK, npc, UO
    )
    # out[n, co, 4b+j, 2u+v]
    out = blocks.transpose(4, 0, 3, 1, 5, 2).reshape(npc, CO, HO, WO)
    return np.ascontiguousarray(out)


def _body(
    nc,
    x,
    o,
    st,
    xpool,
    opool,
    ppool,
    npc,
    do_load=True,
    do_mm=True,
    do_copy=True,
    do_store=True,
    xfix=None,
    obfix=None,
):
    import concourse.mybir as mybir

    f32 = mybir.dt.float32
    bf16 = mybir.dt.bfloat16

    chunk_starts = list(range(0, NBLK, CB))

    def issue_load(idx):
        cs = chunk_starts[idx]
        nblk_c = min(CB, NBLK - cs)
        xt = xpool.tile([KP, CB, npc, NU], bf16, tag="xt")
        for k in range(nblk_c):
            leng = (nc.sync, nc.scalar)[(idx + k) % 2 if ALT_RINGS else 0]
            leng.dma_start(
                xt[:, k: k + 1, :, :].rearrange("p b n u -> p (b n u)"),
                x[:, (cs + k) * npc * NU: (cs + k + 1) * npc * NU],
            )
        return xt

    PREFETCH = 2
    xts = {}
    if do_load:
        for i in range(min(PREFETCH, len(chunk_starts))):
            xts[i] = issue_load(i)

    ob = None
    n_store = [0]
    store_q = []

    def flush_store():
        b0, nsb, obt = store_q.pop(0)
        if STORE_GPSIMD:
            seng = nc.gpsimd
        else:
            for k in range(nsb):
                seng = (nc.scalar, nc.sync)[
                    (n_store[0] + k) % 2 if ALT_RINGS else 0
                ]
                seng.dma_start(
                    o[:, (b0 + k) * npc * UO: (b0 + k + 1) * npc * UO],
                    obt[:, k: k + 1, :, :].rearrange("p b n u -> p (b n u)"),
                )
            n_store[0] += 1
            return

    for b in range(NBLK):
        ci_, bb = divmod(b, CB)
        if do_load:
            if bb == 0:
                if ci_ + PREFETCH < len(chunk_starts):
                    xts[ci_ + PREFETCH] = issue_load(ci_ + PREFETCH)
                if ci_ - 1 in xts:
                    del xts[ci_ - 1]
            xt = xts[ci_]
        else:
            xt = xfix
            bb = 0
        sb = b % SB
        if do_copy and sb == 0:
            ob = opool.tile([128, SB, npc, UO], bf16, tag="ob")
        elif not do_copy:
            ob = obfix
        if do_mm:
            if S_MAJOR:
                pss = [
                    ppool.tile([128, NSUB, UO], f32, tag="ps", name=f"ps{ng}")
                    for ng in range(NGRP)
                ]
                for p in range(NPASS):
                    for ng in range(NGRP):
                        n0 = ng * NSUB
                        nc.tensor.matmul(
                            pss[ng][:],
                            st[:, p, :],
                            xt[:, bb, n0: n0 + NSUB, p: p + UO],
                            start=(p == 0),
                            stop=(p == NPASS - 1),
                        )
                if do_copy:
                    for ng in range(NGRP):
                        n0 = ng * NSUB
                        nc.vector.tensor_copy(
                            ob[:, sb, n0: n0 + NSUB, :], pss[ng][:]
                        )
            else:
                for ng0 in range(0, NGRP, 2):
                    n0a, n0b = ng0 * NSUB, (ng0 + 1) * NSUB
                    psa = ppool.tile([128, NSUB, UO], f32, tag="ps")
                    psb = ppool.tile([128, NSUB, UO], f32, tag="ps")
                    for p in range(NPASS):
                        nc.tensor.matmul(
                            psa[:],
                            st[:, p, :],
                            xt[:, bb, n0a: n0a + NSUB, p: p + UO],
                            start=(p == 0),
                            stop=(p == NPASS - 1),
                        )
                        nc.tensor.matmul(
                            psb[:],
                            st[:, p, :],
                            xt[:, bb, n0b: n0b + NSUB, p: p + UO],
                            start=(p == 0),
                            stop=(p == NPASS - 1),
                        )
                    if do_copy:
                        nc.vector.tensor_copy(
                            ob[:, sb, n0a: n0a + NSUB, :], psa[:]
                        )
                        nc.vector.tensor_copy(
                            ob[:, sb, n0b: n0b + NSUB, :], psb[:]
                        )
        if do_store and (sb == SB - 1 or b == NBLK - 1):
            store_q.append((b - sb, sb + 1, ob))
            while len(store_q) > STORE_LAG:
                flush_store()
    if do_store:
        while store_q:
            flush_store()


def build_nc(npc=NPC, reps=1):
    import concourse.mybir as mybir
    import concourse.tile as tile
    from concourse import bacc

    bf16 = mybir.dt.bfloat16

    nc = bacc.Bacc(None, target_bir_lowering=False)
    x = nc.dram_tensor(
        "x", [KP, NBLK * npc * NU], bf16, kind="ExternalInput"
    )
    s = nc.dram_tensor("s", [KP, NPASS, 128], bf16, kind="ExternalInput")
    o = nc.dram_tensor(
        "o", [128, NBLK * npc * UO], bf16, kind="ExternalOutput"
    )

    with tile.TileContext(nc) as tc:
        with (
            tc.tile_pool(name="spool", bufs=1) as spool,
            tc.tile_pool(name="xpool", bufs=4) as xpool,
            tc.tile_pool(name="opool", bufs=6) as opool,
            tc.tile_pool(name="ppool", bufs=8, space="PSUM") as ppool,
        ):
            st = spool.tile([KP, NPASS, 128], bf16)
            nc.sync.dma_start(st[:], s[:, :, :].rearrange("p a m -> p (a m)"))
            for _rep in range(reps):
                _body(nc, x, o, st, xpool, opool, ppool, npc)
    nc.compile()
    return nc


def _timing_shell(
    npc, reps, body_fn, staggered_reset=False, unroll=1, count=True,
    fixtures=True,
):
    """Common For_i timing harness: internal DRAM output + rep counter."""
    import concourse.mybir as mybir
    import concourse.tile as tile
    from concourse import bacc

    f32 = mybir.dt.float32
    bf16 = mybir.dt.bfloat16
    ET = mybir.EngineType

    nc = bacc.Bacc(None, target_bir_lowering=False)
    x = nc.dram_tensor(
        "x", [KP, NBLK * npc * NU], bf16, kind="ExternalInput"
    )
    s = nc.dram_tensor("s", [KP, NPASS, 128], bf16, kind="ExternalInput")
    t = nc.dram_tensor("t", [1, 1], f32, kind="ExternalOutput")

    with tile.TileContext(nc) as tc:
        with (
            tc.tile_pool(name="spool", bufs=1) as spool,
            tc.tile_pool(name="xpool", bufs=4) as xpool,
            tc.tile_pool(name="opool", bufs=6) as opool,
            tc.tile_pool(name="ppool", bufs=8, space="PSUM") as ppool,
            tc.tile_pool(name="dpool", bufs=1, space="DRAM") as dpool,
        ):
            o = dpool.tile([128, NBLK * npc * UO], bf16)
            st = spool.tile([KP, NPASS, 128], bf16)
            nc.sync.dma_start(st[:], s[:, :, :].rearrange("p a m -> p (a m)"))
            if fixtures:
                xfix = spool.tile([KP, 1, npc, NU], bf16, tag="xfix")
                nc.sync.dma_start(
                    xfix[:].rearrange("p b n u -> p (b n u)"),
                    x[:, 0: npc * NU],
                )
                obfix = spool.tile([128, SB, npc, UO], bf16, tag="obfix")
                nc.gpsimd.memset(obfix[:], 0.25)
            else:
                xfix = obfix = None

            tb = spool.tile([1, 1], f32)
            nc.gpsimd.memset(tb[:], 1.0)
            tzero = spool.tile([1, 1], f32)
            nc.gpsimd.memset(tzero[:], 0.0)
            nc.sync.dma_start(t[:, :], tzero[:])

            def body():
                body_fn(nc, x, o, st, xpool, opool, ppool, xfix, obfix)
                if count:
                    nc.gpsimd.dma_start(
                        t[:, :], tb[:], accum_op=mybir.AluOpType.add
                    )

            if reps == 1:
                body()
            else:
                with tc.For_i(
                    0,
                    (reps - 1) // unroll,
                    1,
                    hint_engines=(ET.PE, ET.Activation, ET.DVE, ET.Pool, ET.SP),
                    staggered_reset=staggered_reset,
                ):
                    body()
    nc.compile()
    return nc


def build_nc_timing(reps, npc=NPC):
    def body_fn(nc, x, o, st, xpool, opool, ppool, xfix, obfix):
        _body(nc, x, o, st, xpool, opool, ppool, npc)

    return _timing_shell(npc, reps, body_fn)


def build_nc_micro(which, reps, npc=NPC):
    if which.startswith("u2"):
        which = which[2:]
        unroll = 2
    else:
        unroll = 1
    if which.startswith("sr"):
        which = which[2:]
        stag = True
    else:
        stag = False
    if which.endswith("_nc"):
        which = which[:-3]
        count = False
    else:
        count = True

    flags = {
        "mm": dict(do_load=False, do_copy=False, do_store=False),
        "mmcopy": dict(do_load=False, do_store=False),
        "load": dict(do_mm=False, do_copy=False, do_store=False),
        "store": dict(do_load=False, do_mm=False, do_copy=False),
        "nostore": dict(do_store=False),
        "mcs": dict(do_load=False),
        "lmst": dict(do_copy=False),
        "loadstore": dict(do_mm=False, do_copy=False),
        "full": dict(),
    }[which]

    def body_fn(nc, x, o, st, xpool, opool, ppool, xfix, obfix):
        for _ in range(unroll):
            _body(
                nc, x, o, st, xpool, opool, ppool, npc,
                xfix=xfix, obfix=obfix, **flags,
            )

    return _timing_shell(
        npc, reps, body_fn, staggered_reset=stag, unroll=unroll, count=count,
        fixtures=(which != "full"),
    )


_NC_CACHE = {}


def _get_nc(npc=NPC):
    if npc not in _NC_CACHE:
        _NC_CACHE[npc] = build_nc(npc)
    return _NC_CACHE[npc]


def make_in_maps(x, W):
    wb = (np.sign(W) * _channel_mask()).astype(np.float32)
    S = _build_stationary(wb).transpose(1, 0, 2).astype(_np_bf16())
    S = np.ascontiguousarray(S)  # [KP, NPASS, 128]
    shards = x.reshape(N_CORES, NPC, CI, H, WI)
    return [{"x": _pack_x(shards[i]), "s": S} for i in range(N_CORES)]


def _run(x, W, trace=False):
    from concourse.bass_utils import run_bass_kernel_spmd

    x = np.asarray(x, dtype=np.float32)
    W = np.asarray(W, dtype=np.float32)
    in_maps = make_in_maps(x, W)
    nc = _get_nc()
    res = run_bass_kernel_spmd(
        nc, in_maps, core_ids=list(range(N_CORES)), trace=trace
    )
    out = np.concatenate(
        [_unpack_o(r["o"], NPC) for r in res.results], axis=0
    )
    return out, res


def kernel(x, W):
    out, _ = _run(x, W, trace=False)
    return out
